# revision 4
# baseline (speedup 1.0000x reference)
"""Focal-loss + smooth-L1 loss kernel for TRN2, SPMD over 8 NeuronCores.

Sharding: data-parallel over the batch axis (B=8 -> one batch row per core).
Each core computes, over its 76725 anchors:
  - per-class weighted histogram h[c]   = sum_n [lab_n==c] * (1-pt)^2 * (-logpt)
  - per-class count histogram  cnt[c]   = sum_n [lab_n==c]        (lab=-1 matches nothing)
  - per-partition smooth-L1 partial sums (positives-masked)
via a single streaming pass; the tiny final reduction happens on host.

Layout: anchor n = 600*p + t  (p = SBUF partition, t = free axis).
Tiles of T=75 anchors/partition; classes innermost, padded 81->82 for
even/aligned fp16 access. One-hot indicators (fp16) feed TensorE matmuls
(lhsT=iseq [P,82], rhs=[w0, 1] [P,2]) accumulating both histograms in PSUM.
"""

import numpy as np

import concourse.bass as bass
import concourse.bacc as bacc
import concourse.mybir as mybir
import concourse.tile as tile
from concourse.bass_utils import run_bass_kernel_spmd

F32 = mybir.dt.float32
F16 = mybir.dt.float16
I16 = mybir.dt.int16
AF = mybir.ActivationFunctionType
OP = mybir.AluOpType
AX = mybir.AxisListType

C = 81
CP = 82  # padded classes (even -> 4B-aligned fp16 rows; pad col never matches)


def build_kernel(A, APP, T):
    """A anchors, APP anchors/partition (p<127 full; p=127 has A-127*APP),
    T anchors/partition per tile. Requires APP % T == 0 and
    A - 127*APP == (APP//T - 1)*T  (i.e. last tile is the only partial-P one).
    """
    n_tiles = APP // T
    t_full = A - 127 * APP  # p=127 valid for t < t_full
    assert t_full == (n_tiles - 1) * T, (A, APP, T, t_full)

    nc = bacc.Bacc(None, target_bir_lowering=False)
    conf = nc.dram_tensor("conf", [A, C], F32, kind="ExternalInput")
    loc = nc.dram_tensor("loc", [A, 4], F32, kind="ExternalInput")
    tgt = nc.dram_tensor("tgt", [A, 5], F32, kind="ExternalInput")
    hist = nc.dram_tensor("hist", [C, 2], F32, kind="ExternalOutput")
    locs = nc.dram_tensor("locs", [128, 1], F32, kind="ExternalOutput")

    def dram_ap(h, row_elems, P, t0, width):
        # anchor n = APP*p + t ; element (n, f) at flat n*row_elems + f
        return bass.AP(
            tensor=h[:, :].tensor,
            offset=t0 * row_elems,
            ap=[[APP * row_elems, P], [row_elems, T], [1, width]],
        )

    with tile.TileContext(nc) as tc:
        with (
            tc.tile_pool(name="singles", bufs=1) as singles,
            tc.tile_pool(name="io", bufs=2) as io,
            tc.tile_pool(name="mid", bufs=2) as mid,
            tc.tile_pool(name="small", bufs=2) as small,
            tc.tile_pool(name="psum", bufs=1, space="PSUM") as psum,
        ):
            # constants
            iota_i = singles.tile([128, CP], I16)
            nc.gpsimd.iota(iota_i[:, :], [[1, CP]], channel_multiplier=0)
            iota16 = singles.tile([128, CP], F16)
            nc.vector.tensor_copy(iota16[:, :], iota_i[:, :])
            strip = singles.tile([128, n_tiles], F32)
            nc.vector.memset(strip[:, :], 0.0)

            ph = psum.tile([CP, 2], F32)

            for i in range(n_tiles):
                t0 = i * T
                P = 128 if i < n_tiles - 1 else 127

                conf_t = io.tile([128, T, C], F32, tag="conf")
                nc.sync.dma_start(conf_t[:P], dram_ap(conf, C, P, t0, C))
                tgt_t = io.tile([128, T, 5], F32, tag="tgt")
                nc.sync.dma_start(tgt_t[:P], dram_ap(tgt, 5, P, t0, 5))
                loc_t = io.tile([128, T, 4], F32, tag="loc")
                nc.sync.dma_start(loc_t[:P], dram_ap(loc, 4, P, t0, 4))

                tlab = tgt_t[:P, :, 4:5]  # [P,T,1] f32

                # ---- conf path ----
                e_t = mid.tile([128, T, CP], F16, tag="e")
                nc.gpsimd.memset(e_t[:P, :, C:CP], 0.0)
                nc.scalar.activation(e_t[:P, :, 0:C], conf_t[:P], AF.Exp)
                s_t = small.tile([128, T], F32, tag="s")
                nc.vector.reduce_sum(s_t[:P], e_t[:P], axis=AX.X)

                iseq = mid.tile([128, T, CP], F16, tag="iseq")
                nc.vector.tensor_tensor(
                    iseq[:P],
                    iota16[:P, None, :].broadcast_to([P, T, CP]),
                    tlab.broadcast_to([P, T, CP]),
                    OP.is_equal,
                )
                me = mid.tile([128, T, CP], F16, tag="me")
                nc.vector.tensor_tensor(me[:P], iseq[:P], e_t[:P], OP.mult)
                ea = small.tile([128, T], F32, tag="ea")
                nc.vector.reduce_sum(ea[:P], me[:P], axis=AX.X)

                # per-anchor scalars [P, T]
                eas = small.tile([128, T], F32, tag="eas")
                nc.vector.tensor_scalar_max(eas[:P], ea[:P], 1e-6)
                rs = small.tile([128, T], F32, tag="rs")
                nc.vector.reciprocal(rs[:P], s_t[:P])
                pt = small.tile([128, T], F32, tag="pt")
                nc.vector.tensor_tensor(pt[:P], ea[:P], rs[:P], OP.mult)
                pm1 = small.tile([128, T], F32, tag="pm1")
                nc.vector.tensor_scalar_add(pm1[:P], pt[:P], -1.0)
                usq = small.tile([128, T], F32, tag="usq")
                nc.scalar.activation(usq[:P], pm1[:P], AF.Square)
                lnea = small.tile([128, T], F32, tag="lnea")
                nc.scalar.activation(lnea[:P], eas[:P], AF.Ln)
                lns = small.tile([128, T], F32, tag="lns")
                nc.scalar.activation(lns[:P], s_t[:P], AF.Ln)
                nlp = small.tile([128, T], F32, tag="nlp")
                nc.vector.tensor_tensor(nlp[:P], lns[:P], lnea[:P], OP.subtract)

                wv = small.tile([128, T, 2], F16, tag="wv")
                nc.gpsimd.memset(wv[:P, :, 1:2], 1.0)
                nc.vector.tensor_tensor(wv[:P, :, 0:1], usq[:P, :, None], nlp[:P, :, None], OP.mult)

                for t in range(T):
                    nc.tensor.matmul(
                        ph[:, :],
                        iseq[:P, t, :],
                        wv[:P, t, :],
                        start=(i == 0 and t == 0),
                        stop=(i == n_tiles - 1 and t == T - 1),
                    )

                # ---- loc path ----
                df = small.tile([128, T, 4], F32, tag="df")
                nc.vector.tensor_tensor(df[:P], loc_t[:P], tgt_t[:P, :, 0:4], OP.subtract)
                da = small.tile([128, T, 4], F32, tag="da")
                nc.scalar.activation(da[:P], df[:P], AF.Abs)
                dm = small.tile([128, T, 4], F32, tag="dm")
                nc.vector.tensor_scalar_min(dm[:P], da[:P], 1.0)
                r_t = small.tile([128, T, 4], F32, tag="r")
                nc.vector.tensor_tensor(r_t[:P], da[:P], dm[:P], OP.subtract)
                sl1 = small.tile([128, T, 4], F32, tag="sl1")
                # (dm * 0.5*dm) + r  == 0.5*m^2 + (d - m)
                q_t = small.tile([128, T, 4], F32, tag="q")
                nc.vector.tensor_tensor(q_t[:P], dm[:P], dm[:P], OP.mult)
                nc.vector.scalar_tensor_tensor(sl1[:P], q_t[:P], 0.5, r_t[:P], OP.mult, OP.add)
                pos = small.tile([128, T], F16, tag="pos")
                nc.vector.tensor_scalar(pos[:P], tlab.squeeze(), 0.0, None, OP.is_gt)
                slm = small.tile([128, T, 4], F32, tag="slm")
                nc.vector.tensor_tensor(
                    slm[:P], sl1[:P], pos[:P, :, None].broadcast_to([P, T, 4]), OP.mult
                )
                nc.vector.reduce_sum(strip[:P, i : i + 1], slm[:P], axis=AX.XY)

            # ---- finalize ----
            hc = singles.tile([CP, 2], F32)
            nc.vector.tensor_copy(hc[:, :], ph[:, :])
            nc.sync.dma_start(hist[:, :], hc[0:C, :])
            lacc = singles.tile([128, 1], F32)
            nc.vector.reduce_sum(lacc[:, :], strip[:, :], axis=AX.X)
            nc.sync.dma_start(locs[:, :], lacc[:, :])

    nc.compile()
    return nc


_CACHED = {}


def _get_nc(A, APP, T):
    key = (A, APP, T)
    if key not in _CACHED:
        _CACHED[key] = build_kernel(A, APP, T)
    return _CACHED[key]


def combine_host(hists, locsums, alpha):
    """hists: [ncores, 81, 2]; locsums: [ncores, 128, 1]; alpha: [81]."""
    h = hists[:, :, 0].sum(axis=0).astype(np.float64)
    cnt = hists[:, :, 1].sum(axis=0).astype(np.float64)
    alpha = alpha.astype(np.float64)
    denom = np.clip(alpha * cnt, 1.0, None)
    conf_loss = np.sum(alpha * h / denom)
    num_pos = cnt[1:].sum()
    loc_sum = locsums.astype(np.float64).sum()
    denom_loc = max(num_pos * 4.0, 1.0)
    loc_loss = loc_sum / denom_loc if num_pos > 0 else 0.0
    return np.float32(loc_loss), np.float32(conf_loss)


def kernel(loc_pred, conf_pred, targets, alpha, _trace=False):
    B, A, _ = conf_pred.shape
    assert B == 8 and A == 76725
    nc = _get_nc(A, 600, 75)
    in_maps = [
        {
            "conf": np.ascontiguousarray(conf_pred[b], dtype=np.float32),
            "loc": np.ascontiguousarray(loc_pred[b], dtype=np.float32),
            "tgt": np.ascontiguousarray(targets[b], dtype=np.float32),
        }
        for b in range(B)
    ]
    res = run_bass_kernel_spmd(nc, in_maps, core_ids=list(range(B)), trace=_trace)
    hists = np.stack([r["hist"] for r in res.results])
    locsums = np.stack([r["locs"] for r in res.results])
    out = combine_host(hists, locsums, np.asarray(alpha, dtype=np.float32))
    if _trace:
        return out, res
    return out
